# revision 36
# baseline (speedup 1.0000x reference)
"""Trainium2 Bass kernel for nn_MultiHeadAttention_88330297410289.

Full-input contract: kernel(**inputs) takes the complete tensors
(hidden_states [32,256,2048], Wq/Wk/Wv/Wo [2048,2048], all fp32) and
returns the full output [32,256,2048] fp32.

Strategy: data-parallel over the batch dim across 8 NeuronCores
(4 batches = 1024 tokens per core, no collectives). Per core, all
activations live in transposed [feature, token] layout so every matmul
streams directly from SBUF with no on-chip transposes:

  qT = WqT.T-contract(xT)    (per head-column block, PSUM [128, 512])
  RoPE: rq via SBUF->SBUF partition-shift DMAs,
        q' = qT*cos + rq*sin on DVE (scale 1/sqrt(hd) folded into q tables)
  scoresT[sk,sq] = k'T.T-contract(q'T) per (batch, head)
  expT = exp(scoresT) on ACT (single [128,512] op per batch)
  sums broadcast over partitions via all-ones matmul; reciprocal on DVE
  outT_un[d,sq] = v.T-contract(expT); normalize on DVE -> outT
  y = outT.T-contract(WoT)   (natural [token, feature] output layout)

Matmuls run in bf16 (fp32 PSUM accumulation); weights/x are cast
host-side; y returns as bf16 and is upcast host-side. The 1/sqrt(hd)
attention scale is folded into Wq host-side.

Scheduling: the two HWDGE rings are split by role — the sync ring
carries the wv quad stream, RoPE partition-shift DMAs, wo slice-0 and
y stores, while the scalar ring carries xt, per-head wq/wk prefetched
one head ahead, and later wo slices — so weight prefetch is never
head-of-line blocked behind a DMA that waits on compute. Weight
streams use [P, 4, 512] quad DMAs (one DGE issue per 4 tiles) to stay
under the ~600 ns/issue sequencer rate. Attention for head h-1 is
interleaved INTO head h's projection groups (scores after q-proj,
sums+attnV split around the k-proj groups) so the PE never waits on
the exp/reciprocal chain. V evacuations are ACT-only (a DVE copy there
can queue behind head-0 rope ops and stall the PSUM-bank reuse at the
phase boundary). PSUM: 8 banks = qkps 4 + scores 2 + acc 2; V phase
uses all 8 for pv, ov=0 i-outer (xt streams), ov>=1 tt-outer (full
16-matmul WAR slack per bank).
"""




import numpy as np
import ml_dtypes

bf16 = ml_dtypes.bfloat16

# Problem shape (hardcoded per contract)
B, S, H = 32, 256, 2048
NH, HD = 16, 128
N_CORES = 8
B_LOC = B // N_CORES          # 4 batches per core
T = B_LOC * S                 # 1024 tokens per core
P = 128

_CACHE = {}


def _rope_tables_np(seq_len, head_dim):
    inv_freq = 1.0 / (10000.0 ** (np.arange(0, head_dim, 2, dtype=np.float32) / head_dim))
    t = np.arange(seq_len, dtype=np.float32)
    freqs = np.einsum("i,j->ij", t, inv_freq).astype(np.float32)   # [s, d/2]
    emb = np.concatenate([freqs, freqs], axis=-1)                   # [s, d]
    return np.cos(emb).astype(np.float32), np.sin(emb).astype(np.float32)


def build_nc(nh=NH, t_tok=T, h_dim=H, b_loc=B_LOC, s_len=S):
    import concourse.tile as tile
    from concourse import bacc, mybir
    import bass_rust

    AF = bass_rust.ActivationFunctionType
    from concourse.alu_op_type import AluOpType

    assert nh * HD == h_dim
    IT = h_dim // P               # contraction i-tiles (16)
    TT = t_tok // P               # token 128-tiles (8)
    TS = t_tok // 512             # token 512-slices (2)
    OS = h_dim // 512             # feature 512-slices (4)
    SK = s_len // P               # key 128-tiles per batch (2)
    f32 = mybir.dt.float32
    bft = mybir.dt.bfloat16

    nc = bacc.Bacc("TRN2", target_bir_lowering=False, debug=False, num_devices=N_CORES)

    xt_d = nc.dram_tensor("xt", [P, IT, t_tok], bft, kind="ExternalInput").ap()
    wq_d = nc.dram_tensor("wq", [P, nh, IT, P], bft, kind="ExternalInput").ap()
    wk_d = nc.dram_tensor("wk", [P, nh, IT, P], bft, kind="ExternalInput").ap()
    wv_d = nc.dram_tensor("wv", [P, IT, h_dim], bft, kind="ExternalInput").ap()
    wo_d = nc.dram_tensor("wo", [P, IT, h_dim], bft, kind="ExternalInput").ap()
    cos_d = nc.dram_tensor("cos", [P, 512], f32, kind="ExternalInput").ap()
    sin_d = nc.dram_tensor("sin", [P, 512], f32, kind="ExternalInput").ap()
    ones_d = nc.dram_tensor("ones", [P, P], bft, kind="ExternalInput").ap()
    y_d = nc.dram_tensor("y", [t_tok, h_dim], bft, kind="ExternalOutput").ap()

    HH = P // 2

    with tile.TileContext(nc) as tc:
        with (
            tc.tile_pool(name="consts", bufs=1) as consts,
            tc.tile_pool(name="xtp", bufs=1) as xtp,
            tc.tile_pool(name="vp", bufs=1) as vp,
            tc.tile_pool(name="outp", bufs=1) as outp,
            tc.tile_pool(name="wqp", bufs=3) as wqp,
            tc.tile_pool(name="wkp", bufs=3) as wkp,
            tc.tile_pool(name="wop", bufs=2) as wop,
            tc.tile_pool(name="ysb", bufs=4) as ysb,
        ):
            # HAM warm-up: memset FIRST on the gpsimd queue (ahead of the
            # const DMAs) so the dummy matmuls run inside the initial
            # DMA-wait window and ramp the PE clock to 2.4 GHz for free;
            # real first matmuls otherwise spend ~3.6 us at the 1.2 GHz
            # mid p-state
            warm = consts.tile([P, 512], bft)
            nc.gpsimd.memset(warm[:], 0.0)

            # consts on the (otherwise idle) GpSimd SWDGE queue
            ones_sb = consts.tile([P, P], bft)
            nc.gpsimd.dma_start(ones_sb[:], ones_d)
            cos_sb = consts.tile([P, 512], f32)
            nc.gpsimd.dma_start(cos_sb[:], cos_d)
            sin_sb = consts.tile([P, 512], f32)
            nc.gpsimd.dma_start(sin_sb[:], sin_d)

            xt_sb = xtp.tile([P, IT, t_tok], bft)
            v_sb = vp.tile([P, TT, h_dim], bft)
            outT_sb = outp.tile([P, nh, t_tok], bft)

            # per-head QK weight prefetch, one head of lead
            def issue_head_w(h):
                wq_t = wqp.tile([P, IT, P], bft, name="wq_t")
                nc.scalar.dma_start(wq_t[:], wq_d[:, h])
                wk_t = wkp.tile([P, IT, P], bft, name="wk_t")
                nc.scalar.dma_start(wk_t[:], wk_d[:, h])
                return (wq_t, wk_t)

            # ---- V projection: v[t, o] ----
            # wv arrives as [P, 4, 512] "quad" DMAs (512 KB, one issue per 4
            # i-tiles) to stay under the DGE sequencer issue rate. ov=0 is
            # i-outer so xt streams in; ov>=1 are tt-outer so each PSUM
            # bank's WAR slack is a full 16-matmul group.
            with (
                tc.tile_pool(name="wvp", bufs=3) as wvp,
                tc.tile_pool(name="vps", bufs=1, space="PSUM") as vps,
            ):
                def evac_v(tt, ov, pv_t):
                    # ACT-only: a DVE evacuation here can queue behind the
                    # first head's rope ops (which wait on shift DMAs) and
                    # stall the PSUM-bank reuse chain at the phase boundary
                    nc.scalar.activation(
                        v_sb[:, tt, ov * 512:(ov + 1) * 512], pv_t[:], AF.Copy
                    )

                def wv_quad(q, ov):
                    w = wvp.tile([P, 4, 512], bft, name=f"wvq{q}")
                    nc.sync.dma_start(
                        w[:], wv_d[:, 4 * q:4 * q + 4, ov * 512:(ov + 1) * 512]
                    )
                    return w

                # early xt split across BOTH rings (i=0 chunks + even i on
                # scalar; xt1/xt3 on sync between the wv singles) so the two
                # cold rings deliver the first four i-steps in parallel
                for c in range(2):
                    nc.scalar.dma_start(
                        xt_sb[:, 0, c * 512:(c + 1) * 512],
                        xt_d[:, 0, c * 512:(c + 1) * 512],
                    )
                nc.scalar.dma_start(xt_sb[:, 2], xt_d[:, 2])
                for i in range(4, IT):
                    nc.scalar.dma_start(xt_sb[:, i], xt_d[:, i])

                # ov = 0, i-outer; the first quad is split small-to-large so
                # the first matmul only waits on a 128 KB transfer
                pv = [vps.tile([P, 512], f32, name=f"pv{tt}") for tt in range(TT)]
                for _ in range(4):
                    nc.tensor.matmul(
                        pv[TT - 1][:], warm[:, 0:128], warm[:],
                        start=True, stop=True,
                    )
                wv0a = wvp.tile([P, 512], bft, name="wv0a")
                nc.sync.dma_start(wv0a[:], wv_d[:, 0, 0:512])
                nc.sync.dma_start(xt_sb[:, 1], xt_d[:, 1])
                wv0b = wvp.tile([P, 512], bft, name="wv0b")
                nc.sync.dma_start(wv0b[:], wv_d[:, 1, 0:512])
                nc.sync.dma_start(xt_sb[:, 3], xt_d[:, 3])
                wv0c = wvp.tile([P, 2, 512], bft, name="wv0c")
                nc.sync.dma_start(wv0c[:], wv_d[:, 2:4, 0:512])
                wv_cur = [None] + [wv_quad(q, 0) for q in range(1, 4)]
                wv_head = [wv0a[:], wv0b[:], wv0c[:, 0], wv0c[:, 1]]
                wv_next = []
                for i in range(IT):
                    # ov=1's quads prefetched in the second half of ov=0
                    if i >= 8 and i % 2 == 0:
                        wv_next.append(wv_quad((i - 8) // 2, 1))
                    wsrc = wv_head[i] if i < 4 else wv_cur[i // 4][:, i % 4]
                    for tt in range(TT):
                        nc.tensor.matmul(
                            pv[tt][:],
                            xt_sb[:, i, tt * P:(tt + 1) * P],
                            wsrc,
                            start=(i == 0),
                            stop=(i == IT - 1),
                        )
                        if i == IT - 1:
                            evac_v(tt, 0, pv[tt])
                for ov in range(1, OS):
                    wv_cur = wv_next
                    wv_next = []
                    for tt in range(TT):
                        pv_t = vps.tile([P, 512], f32, name=f"pv{tt}")
                        for i in range(IT):
                            nc.tensor.matmul(
                                pv_t[:],
                                xt_sb[:, i, tt * P:(tt + 1) * P],
                                wv_cur[i // 4][:, i % 4],
                                start=(i == 0),
                                stop=(i == IT - 1),
                            )
                        evac_v(tt, ov, pv_t)
                        if ov + 1 < OS and tt % 2 == 1:
                            wv_next.append(wv_quad(tt // 2, ov + 1))

            pend_w = [issue_head_w(0), issue_head_w(1)]

            # ---- per-head QK projection + RoPE + attention, interleaved ----
            with (
                tc.tile_pool(name="ropep", bufs=4) as ropep,
                tc.tile_pool(name="cbp", bufs=4) as cbp,
                tc.tile_pool(name="mp", bufs=3) as mp,
                tc.tile_pool(name="ep", bufs=4) as ep,
                tc.tile_pool(name="rsp", bufs=2) as rsp,
                tc.tile_pool(name="qkps", bufs=4, space="PSUM") as qkps,
                tc.tile_pool(name="sps", bufs=2, space="PSUM") as sps,
                tc.tile_pool(name="accps", bufs=2, space="PSUM") as accps,
            ):
                def proj_group(w_t, cos_sb, sin_sb, rope, ts2):
                    sl = slice(ts2 * 512, (ts2 + 1) * 512)
                    pq = qkps.tile([P, 512], f32, name="pq")
                    for i in range(IT):
                        nc.tensor.matmul(
                            pq[:],
                            w_t[:, i],
                            xt_sb[:, i, sl],
                            start=(i == 0),
                            stop=(i == IT - 1),
                        )
                    qbf = cbp.tile([P, 512], bft, name="qbf")
                    nc.scalar.activation(qbf[:], pq[:], AF.Copy)
                    # rotate_half via SBUF->SBUF partition-shift DMAs, one
                    # half per HWDGE ring so they issue in parallel
                    # (sign is folded into the sin tables host-side)
                    rq = cbp.tile([P, 512], bft, name="rq")
                    nc.sync.dma_start(rq[0:HH, :], qbf[HH:P, :])
                    nc.scalar.dma_start(rq[HH:P, :], qbf[0:HH, :])
                    m1 = mp.tile([P, 512], f32, name="m1")
                    nc.vector.tensor_tensor(m1[:], pq[:], cos_sb[:], AluOpType.mult)
                    m2 = mp.tile([P, 512], f32, name="m2")
                    nc.vector.tensor_tensor(m2[:], rq[:], sin_sb[:], AluOpType.mult)
                    nc.vector.tensor_tensor(rope[:, sl], m1[:], m2[:], AluOpType.add)

                def attn_scores(ropes, b):
                    q_rope, k_rope = ropes
                    bs = slice(b * s_len, (b + 1) * s_len)
                    pS = sps.tile([P, SK, s_len], f32, name="pS")
                    for sk in range(SK):
                        nc.tensor.matmul(
                            pS[:, sk],
                            k_rope[:, b * s_len + sk * P: b * s_len + (sk + 1) * P],
                            q_rope[:, bs],
                            start=True,
                            stop=True,
                        )
                    ebf = ep.tile([P, SK, s_len], bft, name="ebf")
                    nc.scalar.activation(ebf[:], pS[:], AF.Exp)
                    return ebf

                def attn_tail(h, b, ebf):
                    bs = slice(b * s_len, (b + 1) * s_len)
                    # sums and attnV share one PSUM bank (halves of acc)
                    acc = accps.tile([P, 2, s_len], f32, name="acc")
                    for sk in range(SK):
                        nc.tensor.matmul(
                            acc[:, 0], ones_sb[:], ebf[:, sk],
                            start=(sk == 0), stop=(sk == SK - 1),
                        )
                    rsb = rsp.tile([P, s_len], f32, name="rsb")
                    nc.vector.reciprocal_approx_fast(rsb[:], acc[:, 0])
                    for sk in range(SK):
                        nc.tensor.matmul(
                            acc[:, 1],
                            v_sb[:, SK * b + sk, h * P:(h + 1) * P],
                            ebf[:, sk],
                            start=(sk == 0), stop=(sk == SK - 1),
                        )
                    nc.vector.tensor_tensor(
                        outT_sb[:, h, bs], acc[:, 1], rsb[:], AluOpType.mult
                    )

                wo_tiles = {}
                prev = None
                for h in range(nh):
                    wq_t, wk_t = pend_w[h]
                    if h + 2 < nh:
                        pend_w.append(issue_head_w(h + 2))
                    if h == nh - 2:
                        # wo slice 0 prefetch: all 16 issues at the top of
                        # head 14, ahead of that head's shift DMAs on sync
                        for quarter in range(4):
                            wo_tiles.setdefault(0, []).extend(
                                _issue_wo_quarter(nc, wop, wo_d, 0, quarter, bft)
                            )
                    q_rope = ropep.tile([P, t_tok], bft, name="q_rope")
                    k_rope = ropep.tile([P, t_tok], bft, name="k_rope")
                    proj_group(wq_t, cos_sb, sin_sb, q_rope, 0)
                    proj_group(wq_t, cos_sb, sin_sb, q_rope, 1)
                    ebfs = None
                    if prev is not None:
                        ebfs = [attn_scores(prev[1], b) for b in range(b_loc)]
                    proj_group(wk_t, cos_sb, sin_sb, k_rope, 0)
                    if prev is not None:
                        attn_tail(prev[0], 0, ebfs[0])
                        attn_tail(prev[0], 1, ebfs[1])
                    proj_group(wk_t, cos_sb, sin_sb, k_rope, 1)
                    if prev is not None:
                        attn_tail(prev[0], 2, ebfs[2])
                        attn_tail(prev[0], 3, ebfs[3])
                    prev = (h, (q_rope, k_rope))

                # trailing attention for the last head
                ebfs = [attn_scores(prev[1], b) for b in range(b_loc)]
                for b in range(b_loc):
                    attn_tail(prev[0], b, ebfs[b])

            # ---- output projection: y[t, o2] ----
            with (
                tc.tile_pool(name="yps", bufs=4, space="PSUM") as yps,
            ):
                for o2 in range(OS):
                    wos = wo_tiles.pop(o2)
                    for tt in range(TT):
                        py = yps.tile([P, 512], f32, name="py")
                        for o in range(IT):
                            nc.tensor.matmul(
                                py[:],
                                outT_sb[:, o, tt * P:(tt + 1) * P],
                                wos[o][:],
                                start=(o == 0),
                                stop=(o == IT - 1),
                            )
                        y_t = ysb.tile([P, 512], f32, name="y_t")
                        if tt % 2 == 0:
                            nc.scalar.activation(y_t[:], py[:], AF.Copy)
                        else:
                            nc.vector.tensor_copy(y_t[:], py[:])
                        nc.sync.dma_start(
                            y_d[tt * P:(tt + 1) * P, o2 * 512:(o2 + 1) * 512], y_t[:]
                        )
                        # next wo slice, 4 issues at a time between copies
                        if o2 + 1 < OS and tt < 4:
                            wo_tiles.setdefault(o2 + 1, []).extend(
                                _issue_wo_quarter(
                                    nc, wop, wo_d, o2 + 1, tt, bft, engine=nc.scalar
                                )
                            )

    nc.compile()
    return nc


def _issue_wo_quarter(nc, wop, wo_d, o2, quarter, bft, engine=None):
    t = wop.tile([P, 4, 512], bft, name=f"woq{quarter}")
    (engine or nc.sync).dma_start(
        t[:], wo_d[:, 4 * quarter:4 * quarter + 4, o2 * 512:(o2 + 1) * 512]
    )
    return [t]


def _host_prep(hidden_states, Wq, Wk, Wv, Wo):
    """Host-side sharding + layout prep. Returns per-core in_maps."""
    x = np.asarray(hidden_states, dtype=np.float32).reshape(B * S, H)

    # weights: transposed + tiled layouts, cast to bf16
    WqT = np.ascontiguousarray(np.asarray(Wq).T)   # [i, o]
    WkT = np.ascontiguousarray(np.asarray(Wk).T)
    WvT = np.ascontiguousarray(np.asarray(Wv).T)
    WoT = np.ascontiguousarray(np.asarray(Wo).T)
    IT = H // P
    # per-head column blocks: [P(p), nh, IT, P(o-within-head)]
    # 1/sqrt(hd) attention scale folded into Wq host-side
    scale = np.float32(HD ** -0.5)
    wq_h = np.ascontiguousarray(
        (WqT * scale).reshape(IT, P, NH, HD).transpose(1, 2, 0, 3)
    ).astype(bf16)
    wk_h = np.ascontiguousarray(
        WkT.reshape(IT, P, NH, HD).transpose(1, 2, 0, 3)
    ).astype(bf16)
    # plain i-tiled: [P, IT, H]
    wv_h = np.ascontiguousarray(WvT.reshape(IT, P, H).transpose(1, 0, 2)).astype(bf16)
    wo_h = np.ascontiguousarray(WoT.reshape(IT, P, H).transpose(1, 0, 2)).astype(bf16)

    cos, sin = _rope_tables_np(S, HD)              # [s, d]
    cosT = np.ascontiguousarray(cos.T)             # [d, s]
    sinT = np.ascontiguousarray(sin.T)
    # rotate-half sign folded into sin: rq[d] = q[(d+64)%128], sign -1 for d<64
    sgn = np.where(np.arange(HD) < HD // 2, -1.0, 1.0).astype(np.float32)[:, None]
    sinT = sinT * sgn
    cos = np.tile(cosT, (1, 2)).astype(np.float32)   # [128, 512]
    sin = np.tile(sinT, (1, 2)).astype(np.float32)
    ones = np.ones((P, P), np.float32).astype(bf16)

    shared = {
        "wq": wq_h, "wk": wk_h, "wv": wv_h, "wo": wo_h,
        "cos": cos, "sin": sin,
        "ones": ones,
    }
    in_maps = []
    for c in range(N_CORES):
        xc = x[c * T:(c + 1) * T]                   # [T, H]
        xTc = np.ascontiguousarray(xc.T).astype(bf16)  # [H, T]
        xt = np.ascontiguousarray(
            xTc.reshape(IT, P, T).transpose(1, 0, 2)
        )                                           # [P, IT, T]
        in_maps.append({"xt": xt, **shared})
    return in_maps


def _run(hidden_states, Wq, Wk, Wv, Wo, **spmd_kwargs):
    from concourse import bass_utils

    if "nc" not in _CACHE:
        _CACHE["nc"] = build_nc()
    nc = _CACHE["nc"]

    in_maps = _host_prep(hidden_states, Wq, Wk, Wv, Wo)
    res = bass_utils.run_bass_kernel_spmd(
        nc, in_maps, core_ids=list(range(N_CORES)), **spmd_kwargs
    )
    y = np.concatenate([r["y"] for r in res.results], axis=0)  # [B*S, H]
    return y.reshape(B, S, H).astype(np.float32), res


def kernel(hidden_states, Wq, Wk, Wv, Wo):
    y, _ = _run(hidden_states, Wq, Wk, Wv, Wo)
    return y


def run_traced(hidden_states, Wq, Wk, Wv, Wo):
    """Like kernel(), but captures an NTFF profile; returns (y, BassKernelResults)."""
    return _run(hidden_states, Wq, Wk, Wv, Wo, trace=True)


# revision 37
# speedup vs baseline: 1.0093x; 1.0093x over previous
"""Trainium2 Bass kernel for nn_MultiHeadAttention_88330297410289.

Full-input contract: kernel(**inputs) takes the complete tensors
(hidden_states [32,256,2048], Wq/Wk/Wv/Wo [2048,2048], all fp32) and
returns the full output [32,256,2048] fp32.

Strategy: data-parallel over the batch dim across 8 NeuronCores
(4 batches = 1024 tokens per core, no collectives). Per core, all
activations live in transposed [feature, token] layout so every matmul
streams directly from SBUF with no on-chip transposes:

  qT = WqT.T-contract(xT)    (per head-column block, PSUM [128, 512])
  RoPE: rq via SBUF->SBUF partition-shift DMAs,
        q' = qT*cos + rq*sin on DVE (scale 1/sqrt(hd) folded into q tables)
  scoresT[sk,sq] = k'T.T-contract(q'T) per (batch, head)
  expT = exp(scoresT) on ACT (single [128,512] op per batch)
  sums broadcast over partitions via all-ones matmul; reciprocal on DVE
  outT_un[d,sq] = v.T-contract(expT); normalize on DVE -> outT
  y = outT.T-contract(WoT)   (natural [token, feature] output layout)

Matmuls run in bf16 (fp32 PSUM accumulation); weights/x are cast
host-side; y returns as bf16 and is upcast host-side. The 1/sqrt(hd)
attention scale is folded into Wq host-side.

Scheduling: the two HWDGE rings are split by role — the sync ring
carries the wv quad stream, RoPE partition-shift DMAs, wo slice-0 and
y stores, while the scalar ring carries xt, per-head wq/wk prefetched
one head ahead, and later wo slices — so weight prefetch is never
head-of-line blocked behind a DMA that waits on compute. Weight
streams use [P, 4, 512] quad DMAs (one DGE issue per 4 tiles) to stay
under the ~600 ns/issue sequencer rate. Attention for head h-1 is
interleaved INTO head h's projection groups (scores after q-proj,
sums+attnV split around the k-proj groups) so the PE never waits on
the exp/reciprocal chain. V evacuations are ACT-only (a DVE copy there
can queue behind head-0 rope ops and stall the PSUM-bank reuse at the
phase boundary). PSUM: 8 banks = qkps 4 + scores 2 + acc 2; V phase
uses all 8 for pv, ov=0 i-outer (xt streams), ov>=1 tt-outer (full
16-matmul WAR slack per bank).
"""

import numpy as np
import ml_dtypes

bf16 = ml_dtypes.bfloat16

# Problem shape (hardcoded per contract)
B, S, H = 32, 256, 2048
NH, HD = 16, 128
N_CORES = 8
B_LOC = B // N_CORES          # 4 batches per core
T = B_LOC * S                 # 1024 tokens per core
P = 128

_CACHE = {}


def _rope_tables_np(seq_len, head_dim):
    inv_freq = 1.0 / (10000.0 ** (np.arange(0, head_dim, 2, dtype=np.float32) / head_dim))
    t = np.arange(seq_len, dtype=np.float32)
    freqs = np.einsum("i,j->ij", t, inv_freq).astype(np.float32)   # [s, d/2]
    emb = np.concatenate([freqs, freqs], axis=-1)                   # [s, d]
    return np.cos(emb).astype(np.float32), np.sin(emb).astype(np.float32)


def build_nc(nh=NH, t_tok=T, h_dim=H, b_loc=B_LOC, s_len=S):
    import concourse.tile as tile
    from concourse import bacc, mybir
    import bass_rust

    AF = bass_rust.ActivationFunctionType
    from concourse.alu_op_type import AluOpType

    assert nh * HD == h_dim
    IT = h_dim // P               # contraction i-tiles (16)
    TT = t_tok // P               # token 128-tiles (8)
    TS = t_tok // 512             # token 512-slices (2)
    OS = h_dim // 512             # feature 512-slices (4)
    SK = s_len // P               # key 128-tiles per batch (2)
    f32 = mybir.dt.float32
    bft = mybir.dt.bfloat16

    nc = bacc.Bacc("TRN2", target_bir_lowering=False, debug=False, num_devices=N_CORES)

    xt_d = nc.dram_tensor("xt", [P, IT, t_tok], bft, kind="ExternalInput").ap()
    wq_d = nc.dram_tensor("wq", [P, nh, IT, P], bft, kind="ExternalInput").ap()
    wk_d = nc.dram_tensor("wk", [P, nh, IT, P], bft, kind="ExternalInput").ap()
    wv_d = nc.dram_tensor("wv", [P, IT, h_dim], bft, kind="ExternalInput").ap()
    wo_d = nc.dram_tensor("wo", [P, IT, h_dim], bft, kind="ExternalInput").ap()
    cos_d = nc.dram_tensor("cos", [P, 512], f32, kind="ExternalInput").ap()
    sin_d = nc.dram_tensor("sin", [P, 512], f32, kind="ExternalInput").ap()
    ones_d = nc.dram_tensor("ones", [P, P], bft, kind="ExternalInput").ap()
    y_d = nc.dram_tensor("y", [t_tok, h_dim], bft, kind="ExternalOutput").ap()

    HH = P // 2

    with tile.TileContext(nc) as tc:
        with (
            tc.tile_pool(name="consts", bufs=1) as consts,
            tc.tile_pool(name="xtp", bufs=1) as xtp,
            tc.tile_pool(name="vp", bufs=1) as vp,
            tc.tile_pool(name="outp", bufs=1) as outp,
            tc.tile_pool(name="wqp", bufs=3) as wqp,
            tc.tile_pool(name="wkp", bufs=3) as wkp,
            tc.tile_pool(name="wop", bufs=2) as wop,
            tc.tile_pool(name="ysb", bufs=4) as ysb,
        ):
            # HAM warm-up: memset FIRST on the gpsimd queue (ahead of the
            # const DMAs) so the dummy matmuls run inside the initial
            # DMA-wait window and ramp the PE clock to 2.4 GHz for free;
            # real first matmuls otherwise spend ~3.6 us at the 1.2 GHz
            # mid p-state
            warm = consts.tile([P, 512], bft)
            nc.gpsimd.memset(warm[:], 0.0)

            # consts on the (otherwise idle) GpSimd SWDGE queue
            ones_sb = consts.tile([P, P], bft)
            nc.gpsimd.dma_start(ones_sb[:], ones_d)
            cos_sb = consts.tile([P, 512], f32)
            nc.gpsimd.dma_start(cos_sb[:], cos_d)
            sin_sb = consts.tile([P, 512], f32)
            nc.gpsimd.dma_start(sin_sb[:], sin_d)

            xt_sb = xtp.tile([P, IT, t_tok], bft)
            v_sb = vp.tile([P, TT, h_dim], bft)
            outT_sb = outp.tile([P, nh, t_tok], bft)

            # per-head QK weight prefetch, one head of lead
            def issue_head_w(h):
                wq_t = wqp.tile([P, IT, P], bft, name="wq_t")
                nc.scalar.dma_start(wq_t[:], wq_d[:, h])
                wk_t = wkp.tile([P, IT, P], bft, name="wk_t")
                nc.scalar.dma_start(wk_t[:], wk_d[:, h])
                return (wq_t, wk_t)

            # ---- V projection: v[t, o] ----
            # wv arrives as [P, 4, 512] "quad" DMAs (512 KB, one issue per 4
            # i-tiles) to stay under the DGE sequencer issue rate. ov=0 is
            # i-outer so xt streams in; ov>=1 are tt-outer so each PSUM
            # bank's WAR slack is a full 16-matmul group.
            with (
                tc.tile_pool(name="wvp", bufs=3) as wvp,
                tc.tile_pool(name="vps", bufs=1, space="PSUM") as vps,
            ):
                def evac_v(tt, ov, pv_t):
                    # ACT-only: a DVE evacuation here can queue behind the
                    # first head's rope ops (which wait on shift DMAs) and
                    # stall the PSUM-bank reuse chain at the phase boundary
                    nc.scalar.activation(
                        v_sb[:, tt, ov * 512:(ov + 1) * 512], pv_t[:], AF.Copy
                    )

                def wv_quad(q, ov):
                    w = wvp.tile([P, 4, 512], bft, name=f"wvq{q}")
                    nc.sync.dma_start(
                        w[:], wv_d[:, 4 * q:4 * q + 4, ov * 512:(ov + 1) * 512]
                    )
                    return w

                # early xt split across BOTH rings (i=0 chunks + even i on
                # scalar; xt1/xt3 on sync between the wv singles) so the two
                # cold rings deliver the first four i-steps in parallel
                for c in range(2):
                    nc.scalar.dma_start(
                        xt_sb[:, 0, c * 512:(c + 1) * 512],
                        xt_d[:, 0, c * 512:(c + 1) * 512],
                    )
                nc.scalar.dma_start(xt_sb[:, 2], xt_d[:, 2])
                for i in range(4, IT):
                    nc.scalar.dma_start(xt_sb[:, i], xt_d[:, i])

                # ov = 0, i-outer; the first quad is split small-to-large so
                # the first matmul only waits on a 128 KB transfer
                pv = [vps.tile([P, 512], f32, name=f"pv{tt}") for tt in range(TT)]
                for _ in range(4):
                    nc.tensor.matmul(
                        pv[TT - 1][:], warm[:, 0:128], warm[:],
                        start=True, stop=True,
                    )
                wv0a = wvp.tile([P, 512], bft, name="wv0a")
                nc.sync.dma_start(wv0a[:], wv_d[:, 0, 0:512])
                nc.sync.dma_start(xt_sb[:, 1], xt_d[:, 1])
                wv0b = wvp.tile([P, 512], bft, name="wv0b")
                nc.sync.dma_start(wv0b[:], wv_d[:, 1, 0:512])
                nc.sync.dma_start(xt_sb[:, 3], xt_d[:, 3])
                wv0c = wvp.tile([P, 2, 512], bft, name="wv0c")
                nc.sync.dma_start(wv0c[:], wv_d[:, 2:4, 0:512])
                wv_cur = [None] + [wv_quad(q, 0) for q in range(1, 4)]
                wv_head = [wv0a[:], wv0b[:], wv0c[:, 0], wv0c[:, 1]]
                wv_next = []
                for i in range(IT):
                    # ov=1's quads prefetched in the second half of ov=0
                    if i >= 8 and i % 2 == 0:
                        wv_next.append(wv_quad((i - 8) // 2, 1))
                    wsrc = wv_head[i] if i < 4 else wv_cur[i // 4][:, i % 4]
                    for tt in range(TT):
                        nc.tensor.matmul(
                            pv[tt][:],
                            xt_sb[:, i, tt * P:(tt + 1) * P],
                            wsrc,
                            start=(i == 0),
                            stop=(i == IT - 1),
                        )
                        if i == IT - 1:
                            evac_v(tt, 0, pv[tt])
                for ov in range(1, OS):
                    wv_cur = wv_next
                    wv_next = []
                    for tt in range(TT):
                        pv_t = vps.tile([P, 512], f32, name=f"pv{tt}")
                        for i in range(IT):
                            nc.tensor.matmul(
                                pv_t[:],
                                xt_sb[:, i, tt * P:(tt + 1) * P],
                                wv_cur[i // 4][:, i % 4],
                                start=(i == 0),
                                stop=(i == IT - 1),
                            )
                        evac_v(tt, ov, pv_t)
                        if ov + 1 < OS and tt % 2 == 1:
                            wv_next.append(wv_quad(tt // 2, ov + 1))

            pend_w = [issue_head_w(0), issue_head_w(1)]

            # ---- per-head QK projection + RoPE + attention, interleaved ----
            with (
                tc.tile_pool(name="ropep", bufs=4) as ropep,
                tc.tile_pool(name="cbp", bufs=4) as cbp,
                tc.tile_pool(name="mp", bufs=3) as mp,
                tc.tile_pool(name="ep", bufs=4) as ep,
                tc.tile_pool(name="rsp", bufs=2) as rsp,
                tc.tile_pool(name="qkps", bufs=4, space="PSUM") as qkps,
                tc.tile_pool(name="sps", bufs=2, space="PSUM") as sps,
                tc.tile_pool(name="accps", bufs=2, space="PSUM") as accps,
            ):
                def proj_group(w_t, cos_sb, sin_sb, rope, ts2):
                    sl = slice(ts2 * 512, (ts2 + 1) * 512)
                    pq = qkps.tile([P, 512], f32, name="pq")
                    for i in range(IT):
                        nc.tensor.matmul(
                            pq[:],
                            w_t[:, i],
                            xt_sb[:, i, sl],
                            start=(i == 0),
                            stop=(i == IT - 1),
                        )
                    qbf = cbp.tile([P, 512], bft, name="qbf")
                    nc.scalar.activation(qbf[:], pq[:], AF.Copy)
                    # rotate_half via SBUF->SBUF partition-shift DMAs, one
                    # half per HWDGE ring so they issue in parallel
                    # (sign is folded into the sin tables host-side)
                    rq = cbp.tile([P, 512], bft, name="rq")
                    nc.sync.dma_start(rq[0:HH, :], qbf[HH:P, :])
                    nc.scalar.dma_start(rq[HH:P, :], qbf[0:HH, :])
                    m1 = mp.tile([P, 512], f32, name="m1")
                    nc.vector.tensor_tensor(m1[:], pq[:], cos_sb[:], AluOpType.mult)
                    m2 = mp.tile([P, 512], f32, name="m2")
                    nc.vector.tensor_tensor(m2[:], rq[:], sin_sb[:], AluOpType.mult)
                    nc.vector.tensor_tensor(rope[:, sl], m1[:], m2[:], AluOpType.add)

                def attn_scores(ropes, b):
                    q_rope, k_rope = ropes
                    bs = slice(b * s_len, (b + 1) * s_len)
                    pS = sps.tile([P, SK, s_len], f32, name="pS")
                    for sk in range(SK):
                        nc.tensor.matmul(
                            pS[:, sk],
                            k_rope[:, b * s_len + sk * P: b * s_len + (sk + 1) * P],
                            q_rope[:, bs],
                            start=True,
                            stop=True,
                        )
                    ebf = ep.tile([P, SK, s_len], bft, name="ebf")
                    nc.scalar.activation(ebf[:], pS[:], AF.Exp)
                    return ebf

                def attn_tail(h, b, ebf):
                    bs = slice(b * s_len, (b + 1) * s_len)
                    # sums and attnV share one PSUM bank (halves of acc)
                    acc = accps.tile([P, 2, s_len], f32, name="acc")
                    for sk in range(SK):
                        nc.tensor.matmul(
                            acc[:, 0], ones_sb[:], ebf[:, sk],
                            start=(sk == 0), stop=(sk == SK - 1),
                        )
                    rsb = rsp.tile([P, s_len], f32, name="rsb")
                    nc.vector.reciprocal_approx_fast(rsb[:], acc[:, 0])
                    for sk in range(SK):
                        nc.tensor.matmul(
                            acc[:, 1],
                            v_sb[:, SK * b + sk, h * P:(h + 1) * P],
                            ebf[:, sk],
                            start=(sk == 0), stop=(sk == SK - 1),
                        )
                    nc.vector.tensor_tensor(
                        outT_sb[:, h, bs], acc[:, 1], rsb[:], AluOpType.mult
                    )

                wo_tiles = {}
                prev = None
                for h in range(nh):
                    wq_t, wk_t = pend_w[h]
                    if h + 2 < nh:
                        pend_w.append(issue_head_w(h + 2))
                    if h == nh - 2:
                        # wo slice 0 prefetch: all 16 issues at the top of
                        # head 14, ahead of that head's shift DMAs on sync
                        for quarter in range(4):
                            wo_tiles.setdefault(0, []).extend(
                                _issue_wo_quarter(nc, wop, wo_d, 0, quarter, bft)
                            )
                    q_rope = ropep.tile([P, t_tok], bft, name="q_rope")
                    k_rope = ropep.tile([P, t_tok], bft, name="k_rope")
                    proj_group(wq_t, cos_sb, sin_sb, q_rope, 0)
                    proj_group(wq_t, cos_sb, sin_sb, q_rope, 1)
                    ebfs = None
                    if prev is not None:
                        ebfs = [attn_scores(prev[1], b) for b in range(b_loc)]
                    proj_group(wk_t, cos_sb, sin_sb, k_rope, 0)
                    if prev is not None:
                        attn_tail(prev[0], 0, ebfs[0])
                        attn_tail(prev[0], 1, ebfs[1])
                    proj_group(wk_t, cos_sb, sin_sb, k_rope, 1)
                    if prev is not None:
                        attn_tail(prev[0], 2, ebfs[2])
                        attn_tail(prev[0], 3, ebfs[3])
                    prev = (h, (q_rope, k_rope))

                # trailing attention for the last head
                ebfs = [attn_scores(prev[1], b) for b in range(b_loc)]
                for b in range(b_loc):
                    attn_tail(prev[0], b, ebfs[b])

            # ---- output projection: y[t, o2] ----
            with (
                tc.tile_pool(name="yps", bufs=4, space="PSUM") as yps,
            ):
                for o2 in range(OS):
                    wos = wo_tiles.pop(o2)
                    for tt in range(TT):
                        py = yps.tile([P, 512], f32, name="py")
                        for o in range(IT):
                            nc.tensor.matmul(
                                py[:],
                                outT_sb[:, o, tt * P:(tt + 1) * P],
                                wos[o][:],
                                start=(o == 0),
                                stop=(o == IT - 1),
                            )
                        y_t = ysb.tile([P, 512], f32, name="y_t")
                        if tt % 2 == 0:
                            nc.scalar.activation(y_t[:], py[:], AF.Copy)
                        else:
                            nc.vector.tensor_copy(y_t[:], py[:])
                        nc.sync.dma_start(
                            y_d[tt * P:(tt + 1) * P, o2 * 512:(o2 + 1) * 512], y_t[:]
                        )
                        # next wo slice, 4 issues at a time between copies
                        if o2 + 1 < OS and tt < 4:
                            wo_tiles.setdefault(o2 + 1, []).extend(
                                _issue_wo_quarter(
                                    nc, wop, wo_d, o2 + 1, tt, bft, engine=nc.scalar
                                )
                            )

    nc.compile()
    return nc


def _issue_wo_quarter(nc, wop, wo_d, o2, quarter, bft, engine=None):
    t = wop.tile([P, 4, 512], bft, name=f"woq{quarter}")
    (engine or nc.sync).dma_start(
        t[:], wo_d[:, 4 * quarter:4 * quarter + 4, o2 * 512:(o2 + 1) * 512]
    )
    return [t]


def _host_prep(hidden_states, Wq, Wk, Wv, Wo):
    """Host-side sharding + layout prep. Returns per-core in_maps."""
    x = np.asarray(hidden_states, dtype=np.float32).reshape(B * S, H)

    # weights: transposed + tiled layouts, cast to bf16
    WqT = np.ascontiguousarray(np.asarray(Wq).T)   # [i, o]
    WkT = np.ascontiguousarray(np.asarray(Wk).T)
    WvT = np.ascontiguousarray(np.asarray(Wv).T)
    WoT = np.ascontiguousarray(np.asarray(Wo).T)
    IT = H // P
    # per-head column blocks: [P(p), nh, IT, P(o-within-head)]
    # 1/sqrt(hd) attention scale folded into Wq host-side
    scale = np.float32(HD ** -0.5)
    wq_h = np.ascontiguousarray(
        (WqT * scale).reshape(IT, P, NH, HD).transpose(1, 2, 0, 3)
    ).astype(bf16)
    wk_h = np.ascontiguousarray(
        WkT.reshape(IT, P, NH, HD).transpose(1, 2, 0, 3)
    ).astype(bf16)
    # plain i-tiled: [P, IT, H]
    wv_h = np.ascontiguousarray(WvT.reshape(IT, P, H).transpose(1, 0, 2)).astype(bf16)
    wo_h = np.ascontiguousarray(WoT.reshape(IT, P, H).transpose(1, 0, 2)).astype(bf16)

    cos, sin = _rope_tables_np(S, HD)              # [s, d]
    cosT = np.ascontiguousarray(cos.T)             # [d, s]
    sinT = np.ascontiguousarray(sin.T)
    # rotate-half sign folded into sin: rq[d] = q[(d+64)%128], sign -1 for d<64
    sgn = np.where(np.arange(HD) < HD // 2, -1.0, 1.0).astype(np.float32)[:, None]
    sinT = sinT * sgn
    cos = np.tile(cosT, (1, 2)).astype(np.float32)   # [128, 512]
    sin = np.tile(sinT, (1, 2)).astype(np.float32)
    ones = np.ones((P, P), np.float32).astype(bf16)

    shared = {
        "wq": wq_h, "wk": wk_h, "wv": wv_h, "wo": wo_h,
        "cos": cos, "sin": sin,
        "ones": ones,
    }
    in_maps = []
    for c in range(N_CORES):
        xc = x[c * T:(c + 1) * T]                   # [T, H]
        xTc = np.ascontiguousarray(xc.T).astype(bf16)  # [H, T]
        xt = np.ascontiguousarray(
            xTc.reshape(IT, P, T).transpose(1, 0, 2)
        )                                           # [P, IT, T]
        in_maps.append({"xt": xt, **shared})
    return in_maps


def _run(hidden_states, Wq, Wk, Wv, Wo, **spmd_kwargs):
    from concourse import bass_utils

    if "nc" not in _CACHE:
        _CACHE["nc"] = build_nc()
    nc = _CACHE["nc"]

    in_maps = _host_prep(hidden_states, Wq, Wk, Wv, Wo)
    res = bass_utils.run_bass_kernel_spmd(
        nc, in_maps, core_ids=list(range(N_CORES)), **spmd_kwargs
    )
    y = np.concatenate([r["y"] for r in res.results], axis=0)  # [B*S, H]
    return y.reshape(B, S, H).astype(np.float32), res


def kernel(hidden_states, Wq, Wk, Wv, Wo):
    y, _ = _run(hidden_states, Wq, Wk, Wv, Wo)
    return y


def run_traced(hidden_states, Wq, Wk, Wv, Wo):
    """Like kernel(), but captures an NTFF profile; returns (y, BassKernelResults)."""
    return _run(hidden_states, Wq, Wk, Wv, Wo, trace=True)
